# revision 24
# baseline (speedup 1.0000x reference)
"""TRN2 Bass kernel for nn_ChartOperator (sparse_attention).

Math (B=4, N=4096, PD=1024, D=16, S=64, ALL=1024):
  P = x @ W_r + b_r
  L = P[..., :ALL].reshape(n, D, S); R = P[..., ALL:].reshape(n, D, S)
  w = softmax_a(L)
  Q[n, d, s] = sum_{a<d} w[n,a,s] * R[n+a+1, d-1-a, s]
  (last D rows of each batch: Q[t+d>=16] zeroed)
  out = Q.reshape(n, ALL) @ W_w + b_w

Sharding: 8 cores, data-parallel over flattened (B*N) rows, 2048 rows/core
plus a 16-row forward halo (zero-padded at batch boundaries; the affected
outputs are exactly the masked ones).

Per-core pipeline (bf16 matmuls, fp32 PSUM), software-pipelined so the
banded-MAC + writer chunklets interleave with the reader supertiles:
  PE order: rd(0) rd(2) rd(1) mac(0..2) rd(3,R-first) rd(halo) mac(3..7)
  1. Reader computes P TRANSPOSED: psum[(d2,s64), n512] = W_r_slab.T @ xT.
  2. ACT exp/copy writes banded-MAC layout:
     e chain  et_all[(g2,s64), a16, n1024]  (g: row-blocks 0-7 / 8-15)
     r chain  rt_all[(g2,s64), c16, n1040]  (blocks 0-8 / 8-16 incl halo)
     rt1_all = rt_all shifted by 1 col (so every banded product has a
     4B-aligned operand -> DVE 2x mode).
  3. softmax denom Z on GpSimd (strided reduce); DVE recip + normalize.
  4. banded products p[s, c, n] = w[s,a,n]*r[s,c,n+a+1]: even-a via rt1
     (DVE 2x), a in {0,2,4} offloaded to GpSimd.
  5. PE identity-matmuls accumulate products into PSUM Q[(g,s), d, n]
     with shrinking d-windows (d = a+c+1)
  6. ACT/DVE strided copies Q -> qt[(dsub2,s64), k8, n128] bf16
  7. writer matmuls out[n128, 1024] = qt.T @ W_w + b_w -> DMA out
"""
import numpy as np
import ml_dtypes
from contextlib import ExitStack

import concourse.bass as bass
import concourse.tile as tile
from concourse import bacc, mybir
from concourse import bass_utils

BF16 = mybir.dt.bfloat16
F32 = mybir.dt.float32
bfnp = ml_dtypes.bfloat16

B, N, PD = 4, 4096, 1024
D, S = 16, 64
ALL = D * S
ROWS_PER_CORE = 2048
HALO = 16
NROWS = ROWS_PER_CORE + HALO   # 2064
RTC = 1040                     # rt chain length (needs cols <= 1039)
NCP = 8
GP_A = (0, 2)                  # product shifts computed on GpSimd

_cache = {}


def _build(debug=False):
    nc = bacc.Bacc("TRN2", target_bir_lowering=False, debug=False, num_devices=8)

    xT_d = nc.dram_tensor("xT", [8, 128, NROWS], BF16, kind="ExternalInput").ap()
    wr_d = nc.dram_tensor("wr", [16, 128, 1024], BF16, kind="ExternalInput").ap()
    ww_d = nc.dram_tensor("ww", [8, 128, 1024], BF16, kind="ExternalInput").ap()
    br_d = nc.dram_tensor("br", [128, 16], F32, kind="ExternalInput").ap()
    ident_d = nc.dram_tensor("ident", [128, 128], BF16, kind="ExternalInput").ap()
    qmask_d = nc.dram_tensor("qmask", [128, 8, 128], BF16, kind="ExternalInput").ap()
    out_d = nc.dram_tensor("out", [16, 128, 1024], F32, kind="ExternalOutput").ap()

    AF = mybir.ActivationFunctionType

    with tile.TileContext(nc) as tc, ExitStack() as ctx:
        cpool = ctx.enter_context(tc.tile_pool(name="cpool", bufs=1))
        ps512 = ctx.enter_context(tc.tile_pool(name="ps512", bufs=2, space="PSUM"))
        wps = ctx.enter_context(tc.tile_pool(name="wps", bufs=2, space="PSUM"))
        macp = ctx.enter_context(tc.tile_pool(name="macp", bufs=1, space="PSUM"))
        prodp = ctx.enter_context(tc.tile_pool(name="prodp", bufs=4))
        qtp = ctx.enter_context(tc.tile_pool(name="qtp", bufs=4))
        osbp = ctx.enter_context(tc.tile_pool(name="osbp", bufs=2))

        # --- persistent tiles
        xkj = [cpool.tile([128, 8, 512], BF16, name=f"xkj{j}", tag=f"xkj{j}")
               for j in range(4)]
        xh = cpool.tile([128, 8, HALO], BF16, name="xh", tag="xh")
        wr_sb = cpool.tile([128, 16, 1024], BF16)   # (p, u, ks*128+c)
        ww_sb = cpool.tile([128, 8, 1024], BF16)
        br_sb = cpool.tile([128, 16], F32)
        ident = cpool.tile([128, 128], BF16)
        qmask = cpool.tile([128, 8, 128], BF16)
        et_all = cpool.tile([128, 16, 1024], BF16)   # [(g2,s64), a, n-chain]
        rt_all = cpool.tile([128, 16, RTC], BF16)    # [(g2,s64), c, n-chain]
        rt1_all = cpool.tile([128, 15, RTC], BF16)   # rt shifted by +1 col
        zts = cpool.tile([128, 4, 128], F32)         # denom tree scratch
        zt = cpool.tile([128, 128], F32)             # softmax denom
        rzb = cpool.tile([128, 128], BF16)           # 1/Z bf16

        # --- input DMAs on sync, strictly ordered by first use so the
        # first-supertile weights/activations get the full DMA bandwidth
        nc.sync.dma_start(xkj[0][:], xT_d[:, :, 0:512].rearrange("k p n -> p k n"))
        nc.sync.dma_start(wr_sb[:, 0, :], wr_d[0])
        nc.sync.dma_start(wr_sb[:, 1:4, :], wr_d[1:4].rearrange("u p c -> p u c"))
        nc.sync.dma_start(br_sb[:], br_d[:])
        nc.sync.dma_start(wr_sb[:, 4:8, :], wr_d[4:8].rearrange("u p c -> p u c"))
        nc.sync.dma_start(wr_sb[:, 8:12, :], wr_d[8:12].rearrange("u p c -> p u c"))
        nc.sync.dma_start(wr_sb[:, 12:16, :], wr_d[12:16].rearrange("u p c -> p u c"))
        nc.sync.dma_start(xkj[2][:], xT_d[:, :, 1024:1536].rearrange("k p n -> p k n"))
        nc.sync.dma_start(xkj[1][:], xT_d[:, :, 512:1024].rearrange("k p n -> p k n"))
        nc.sync.dma_start(ident[:], ident_d[:])
        nc.sync.dma_start(xkj[3][:], xT_d[:, :, 1536:2048].rearrange("k p n -> p k n"))
        nc.sync.dma_start(xh[:], xT_d[:, :, 2048:NROWS].rearrange("k p n -> p k n"))
        nc.sync.dma_start(ww_sb[:, 0:4, :], ww_d[0:4].rearrange("k p c -> p k c"))
        nc.sync.dma_start(ww_sb[:, 4:8, :], ww_d[4:8].rearrange("k p c -> p k c"))
        nc.sync.dma_start(qmask[:], qmask_d[:])

        def reader(jj, uorder=None):
            """One 512-row supertile (jj<4) or the 16-row halo (jj==4)."""
            halo = jj == 4
            nwin = HALO if halo else 512
            n0 = jj * 512
            g = 0 if jj < 2 else 1
            xt = xh if halo else xkj[jj]
            for u in (uorder or range(16)):
                is_l = u < 8
                if is_l and halo:
                    continue                  # halo rows: R only
                ps = ps512.tile([128, 512], F32, tag="ps512", name="ps")
                for ks in range(8):
                    nc.tensor.matmul(ps[:, :nwin], wr_sb[:, u, 128 * ks:128 * ks + 128],
                                     xt[:, ks, :nwin],
                                     start=(ks == 0), stop=(ks == 7))
                for dsub in range(2):
                    src = ps[64 * dsub:64 * dsub + 64, :nwin]
                    bias = br_sb[64 * dsub:64 * dsub + 64, u:u + 1]
                    if is_l:
                        a = 2 * u + dsub
                        dst = et_all[64 * g:64 * g + 64, a,
                                     n0 - 1024 * g:n0 - 1024 * g + nwin]
                        nc.scalar.activation(dst, src, AF.Exp, bias=bias)
                    else:
                        c = 2 * (u - 8) + dsub
                        # g0 chain: blocks 0..8+ ; g1 chain: blocks 8..16+
                        if jj < 2:
                            nc.scalar.activation(rt_all[0:64, c, n0:n0 + 512], src,
                                                 AF.Identity, bias=bias)
                        elif jj == 2:   # blocks 8-11: both chains
                            # tiny block-8 tail write goes on vector so the
                            # scalar queue keeps pace with the PE supertile
                            nc.vector.tensor_scalar_add(
                                rt_all[0:64, c, 1024:RTC],
                                ps[64 * dsub:64 * dsub + 64, 0:RTC - 1024], bias)
                            nc.scalar.activation(rt_all[64:128, c, 0:512], src,
                                                 AF.Identity, bias=bias)
                        elif jj == 3:
                            nc.scalar.activation(rt_all[64:128, c, 512:1024], src,
                                                 AF.Identity, bias=bias)
                        else:           # halo block 16
                            nc.scalar.activation(rt_all[64:128, c, 1024:1024 + HALO],
                                                 src, AF.Identity, bias=bias)

        def rt1_copy(lo, hi, eng=None):
            # rt1[:, c, j] = rt[:, c, j+1]
            if eng is None:
                nc.scalar.copy(rt1_all[:, :, lo:hi], rt_all[:, 0:15, lo + 1:hi + 1])
            else:
                eng.tensor_copy(rt1_all[:, :, lo:hi], rt_all[:, 0:15, lo + 1:hi + 1])

        def norm128(w):
            """softmax denom + normalize et over cols [128w, 128w+128)."""
            lo = 128 * w
            e = et_all[:, :, lo:lo + 128]
            nc.vector.tensor_tensor(zts[:], e[:, 0:4, :], e[:, 4:8, :],
                                    op=mybir.AluOpType.add)
            nc.vector.tensor_tensor(zts[:], zts[:], e[:, 8:12, :],
                                    op=mybir.AluOpType.add)
            nc.vector.tensor_tensor(zts[:], zts[:], e[:, 12:16, :],
                                    op=mybir.AluOpType.add)
            nc.vector.tensor_tensor(zts[:, 0:2, :], zts[:, 0:2, :], zts[:, 2:4, :],
                                    op=mybir.AluOpType.add)
            nc.vector.tensor_tensor(zt[:], zts[:, 0, :], zts[:, 1, :],
                                    op=mybir.AluOpType.add)
            nc.vector.reciprocal(zt[:], zt[:])
            nc.vector.tensor_copy(rzb[:], zt[:])
            rz3 = rzb[:].rearrange("p (o n) -> p o n", o=1) \
                .to_broadcast((128, 15, 128))
            ew = et_all[:, 0:15, lo:lo + 128]
            nc.vector.tensor_mul(ew, ew, rz3)

        qts = {}

        def band(cp):
            """Banded MAC for one 128-row chunklet pair -> qt tiles."""
            mp = macp.tile([128, 16, 128], F32, tag="macp", name="mp")
            nc.vector.memset(mp[:, 0, :], 0.0)
            n0 = 128 * cp
            for a in range(15):
                cnt = 15 - a
                p = prodp.tile([128, 15, 128], BF16, tag="prodp", name="p")
                eb = et_all[:, a:a + 1, n0:n0 + 128].to_broadcast((128, cnt, 128))
                eng = nc.gpsimd if a in GP_A else nc.vector
                if a % 2 == 0:
                    # even shift a+1 is 2B-misaligned on rt; use the
                    # pre-shifted rt1 so DVE runs in 2x mode
                    eng.tensor_mul(p[:, 0:cnt, :], eb,
                                   rt1_all[:, 0:cnt, n0 + a:n0 + a + 128])
                else:
                    eng.tensor_mul(p[:, 0:cnt, :], eb,
                                   rt_all[:, 0:cnt, n0 + a + 1:n0 + a + 129])
                for b in range(4):
                    d_lo = max(a + 1, 4 * b)
                    d_hi = 4 * b + 4
                    if d_lo >= d_hi:
                        continue
                    last_a = min(14, 4 * b + 2)
                    nc.tensor.matmul(mp[:, d_lo:d_hi, :], ident[:],
                                     p[:, d_lo - a - 1:d_hi - a - 1, :],
                                     start=(a == 0), stop=(a == last_a))

            for g in range(2):
                qt = qtp.tile([128, 8, 128], BF16, tag="qtp", name="qt")
                for dsub in range(2):
                    csrc = mp[64 * g:64 * g + 64, dsub::2, :]
                    cdst = qt[64 * dsub:64 * dsub + 64, :, :]
                    nc.scalar.copy(cdst, csrc)
                if 8 * g + cp == 15:
                    nc.vector.tensor_mul(qt[:], qt[:], qmask[:])
                qts[(cp, g)] = qt

        def writer(cp):
            """Writer matmuls for chunklet cp (runs one cp behind band)."""
            for g in range(2):
                cb = 8 * g + cp
                qt = qts.pop((cp, g))
                for h in range(2):
                    wp = wps.tile([128, 512], F32, tag="wps", name="wp")
                    for k in range(8):
                        nc.tensor.matmul(wp[:], qt[:, k, :],
                                         ww_sb[:, k, h * 512:(h + 1) * 512],
                                         start=(k == 0), stop=(k == 7))
                    osb = osbp.tile([128, 512], F32, tag="osbp", name="osb")
                    nc.scalar.copy(osb[:], wp[:])
                    nc.sync.dma_start(out_d[cb][:, h * 512:(h + 1) * 512], osb[:])

        # ---------------- interleaved schedule
        reader(0)
        reader(2)
        for w in range(4):
            norm128(w)
        rt1_copy(0, 511, eng=nc.vector)
        reader(1)
        band(0)
        band(1)
        writer(0)
        band(2)
        writer(1)
        # last supertile R-slabs first so rt (and rt1) unblock band(3) early
        reader(3, uorder=list(range(8, 16)))
        rt1_copy(511, 655, eng=nc.vector)
        rt1_copy(655, 1023, eng=nc.vector)
        reader(3, uorder=list(range(0, 8)))
        reader(4)
        band(3)
        writer(2)
        norm128(4)
        band(4)
        writer(3)
        norm128(5)
        band(5)
        writer(4)
        norm128(6)
        band(6)
        writer(5)
        norm128(7)
        rt1_copy(1023, RTC - 1, eng=nc.vector)
        band(7)
        writer(6)
        writer(7)

    nc.compile()
    return nc


def _host_prep(x, W_r, b_r, W_w, b_w):
    """Build the 8 per-core input maps."""
    xf = np.asarray(x, np.float32).reshape(B * N, PD)
    wr = np.asarray(W_r, np.float32).astype(bfnp)
    ww = np.asarray(W_w, np.float32).astype(bfnp)
    br = np.ascontiguousarray(
        np.asarray(b_r, np.float32).reshape(16, 128).T)
    # wr_d[u, p, ks*128+c] = W_r[128*ks + p, 128*u + c]
    wr_t = np.ascontiguousarray(
        wr.reshape(8, 128, 16, 128).transpose(2, 1, 0, 3).reshape(16, 128, 1024))
    ww_t = np.ascontiguousarray(ww.reshape(8, 128, 1024))
    ident = np.eye(128, dtype=np.float32).astype(bfnp)

    in_maps = []
    for c in range(8):
        lo = c * ROWS_PER_CORE
        chunk = np.zeros((NROWS, PD), np.float32)
        chunk[:ROWS_PER_CORE] = xf[lo:lo + ROWS_PER_CORE]
        if c % 2 == 0:
            chunk[ROWS_PER_CORE:] = xf[lo + ROWS_PER_CORE: lo + NROWS]
        # xT[ks, k, n] = chunk[n, 128*ks + k]
        xt = np.ascontiguousarray(
            chunk.astype(bfnp).reshape(NROWS, 8, 128).transpose(1, 2, 0))
        qmask = np.ones((128, 8, 128), np.float32)
        if c % 2 == 1:
            dsub = (np.arange(128)[:, None, None] // 64)
            k = np.arange(8)[None, :, None]
            n = np.arange(128)[None, None, :]
            bad = (n >= 112) & ((n - 112 + 2 * k + dsub) >= 16)
            qmask[np.broadcast_to(bad, (128, 8, 128))] = 0.0
        in_maps.append({
            "xT": xt,
            "wr": wr_t, "ww": ww_t, "br": br,
            "ident": ident, "qmask": qmask.astype(bfnp),
        })
    return in_maps


def kernel(x, W_r, b_r, W_w, b_w):
    if "nc" not in _cache:
        _cache["nc"] = _build()
    nc = _cache["nc"]
    in_maps = _host_prep(x, W_r, b_r, W_w, b_w)
    res = bass_utils.run_bass_kernel_spmd(nc, in_maps, core_ids=list(range(8)))
    out = np.concatenate([r["out"].reshape(ROWS_PER_CORE, ALL)
                          for r in res.results], axis=0)
    out = out.reshape(B, N, ALL).astype(np.float32)
    out += np.asarray(b_w, np.float32).reshape(1, 1, ALL)
    return np.ascontiguousarray(out)


# revision 26
# speedup vs baseline: 1.0344x; 1.0344x over previous
"""TRN2 Bass kernel for nn_ChartOperator (sparse_attention).

Math (B=4, N=4096, PD=1024, D=16, S=64, ALL=1024):
  P = x @ W_r + b_r
  L = P[..., :ALL].reshape(n, D, S); R = P[..., ALL:].reshape(n, D, S)
  w = softmax_a(L)
  Q[n, d, s] = sum_{a<d} w[n,a,s] * R[n+a+1, d-1-a, s]
  (last D rows of each batch: Q[t+d>=16] zeroed)
  out = Q.reshape(n, ALL) @ W_w + b_w

Sharding: 8 cores, data-parallel over flattened (B*N) rows, 2048 rows/core
plus a 16-row forward halo (zero-padded at batch boundaries; the affected
outputs are exactly the masked ones).

Per-core pipeline (bf16 matmuls, fp32 PSUM), software-pipelined so the
banded-MAC + writer chunklets interleave with the reader supertiles:
  PE order: rd(0) rd(2) rd(1) mac(0..2) rd(3,R-first) rd(halo) mac(3..7)
  1. Reader computes P TRANSPOSED: psum[(d2,s64), n512] = W_r_slab.T @ xT.
  2. ACT exp/copy writes banded-MAC layout:
     e chain  et_all[(g2,s64), a16, n1024]  (g: row-blocks 0-7 / 8-15)
     r chain  rt_all[(g2,s64), c16, n1040]  (blocks 0-8 / 8-16 incl halo)
     rt1_all = rt_all shifted by 1 col (so every banded product has a
     4B-aligned operand -> DVE 2x mode).
  3. softmax denom Z on GpSimd (strided reduce); DVE recip + normalize.
  4. banded products p[s, c, n] = w[s,a,n]*r[s,c,n+a+1]: even-a via rt1
     (DVE 2x), a in {0,2,4} offloaded to GpSimd.
  5. PE identity-matmuls accumulate products into PSUM Q[(g,s), d, n]
     with shrinking d-windows (d = a+c+1)
  6. ACT/DVE strided copies Q -> qt[(dsub2,s64), k8, n128] bf16
  7. writer matmuls out[n128, 1024] = qt.T @ W_w + b_w -> DMA out
"""
import numpy as np
import ml_dtypes
from contextlib import ExitStack

import concourse.bass as bass
import concourse.tile as tile
from concourse import bacc, mybir
from concourse import bass_utils

BF16 = mybir.dt.bfloat16
F32 = mybir.dt.float32
bfnp = ml_dtypes.bfloat16

B, N, PD = 4, 4096, 1024
D, S = 16, 64
ALL = D * S
ROWS_PER_CORE = 2048
HALO = 16
NROWS = ROWS_PER_CORE + HALO   # 2064
RTC = 1040                     # rt chain length (needs cols <= 1039)
NCP = 8
GP_A = (0, 2)                  # product shifts computed on GpSimd

_cache = {}


def _build(debug=False):
    nc = bacc.Bacc("TRN2", target_bir_lowering=False, debug=False, num_devices=8)

    xT_d = nc.dram_tensor("xT", [8, 128, NROWS], BF16, kind="ExternalInput").ap()
    wr_d = nc.dram_tensor("wr", [16, 128, 1024], BF16, kind="ExternalInput").ap()
    ww_d = nc.dram_tensor("ww", [8, 128, 1024], BF16, kind="ExternalInput").ap()
    br_d = nc.dram_tensor("br", [128, 16], F32, kind="ExternalInput").ap()
    ident_d = nc.dram_tensor("ident", [128, 128], BF16, kind="ExternalInput").ap()
    qmask_d = nc.dram_tensor("qmask", [128, 8, 128], BF16, kind="ExternalInput").ap()
    out_d = nc.dram_tensor("out", [16, 128, 1024], F32, kind="ExternalOutput").ap()

    AF = mybir.ActivationFunctionType

    with tile.TileContext(nc) as tc, ExitStack() as ctx:
        cpool = ctx.enter_context(tc.tile_pool(name="cpool", bufs=1))
        ps512 = ctx.enter_context(tc.tile_pool(name="ps512", bufs=2, space="PSUM"))
        wps = ctx.enter_context(tc.tile_pool(name="wps", bufs=2, space="PSUM"))
        macp = ctx.enter_context(tc.tile_pool(name="macp", bufs=1, space="PSUM"))
        prodp = ctx.enter_context(tc.tile_pool(name="prodp", bufs=4))
        qtp = ctx.enter_context(tc.tile_pool(name="qtp", bufs=4))
        osbp = ctx.enter_context(tc.tile_pool(name="osbp", bufs=2))

        # --- persistent tiles
        xkj = [cpool.tile([128, 8, 512], BF16, name=f"xkj{j}", tag=f"xkj{j}")
               for j in range(4)]
        xh = cpool.tile([128, 8, HALO], BF16, name="xh", tag="xh")
        wr_sb = cpool.tile([128, 16, 1024], BF16)   # (p, u, ks*128+c)
        ww_sb = cpool.tile([128, 8, 1024], BF16)
        br_sb = cpool.tile([128, 16], F32)
        ident = cpool.tile([128, 128], BF16)
        qmask = cpool.tile([128, 8, 128], BF16)
        et_all = cpool.tile([128, 16, 1024], BF16)   # [(g2,s64), a, n-chain]
        rt_all = cpool.tile([128, 16, RTC], BF16)    # [(g2,s64), c, n-chain]
        rt1_all = cpool.tile([128, 15, RTC], BF16)   # rt shifted by +1 col
        zts = cpool.tile([128, 4, 128], F32)         # denom tree scratch
        zt = cpool.tile([128, 128], F32)             # softmax denom
        rzb = cpool.tile([128, 128], BF16)           # 1/Z bf16

        # --- input DMAs on sync, strictly ordered by first use so the
        # first-supertile weights/activations get the full DMA bandwidth
        nc.sync.dma_start(xkj[0][:], xT_d[:, :, 0:512].rearrange("k p n -> p k n"))
        nc.sync.dma_start(wr_sb[:, 0, :], wr_d[0])
        nc.sync.dma_start(wr_sb[:, 1:4, :], wr_d[1:4].rearrange("u p c -> p u c"))
        nc.sync.dma_start(br_sb[:], br_d[:])
        nc.sync.dma_start(wr_sb[:, 4:8, :], wr_d[4:8].rearrange("u p c -> p u c"))
        nc.sync.dma_start(wr_sb[:, 8:12, :], wr_d[8:12].rearrange("u p c -> p u c"))
        nc.sync.dma_start(wr_sb[:, 12:16, :], wr_d[12:16].rearrange("u p c -> p u c"))
        nc.sync.dma_start(xkj[2][:], xT_d[:, :, 1024:1536].rearrange("k p n -> p k n"))
        nc.sync.dma_start(xkj[1][:], xT_d[:, :, 512:1024].rearrange("k p n -> p k n"))
        nc.sync.dma_start(ident[:], ident_d[:])
        nc.sync.dma_start(xkj[3][:], xT_d[:, :, 1536:2048].rearrange("k p n -> p k n"))
        nc.sync.dma_start(xh[:], xT_d[:, :, 2048:NROWS].rearrange("k p n -> p k n"))
        nc.sync.dma_start(ww_sb[:, 0:4, :], ww_d[0:4].rearrange("k p c -> p k c"))
        nc.sync.dma_start(ww_sb[:, 4:8, :], ww_d[4:8].rearrange("k p c -> p k c"))
        nc.sync.dma_start(qmask[:], qmask_d[:])

        def reader(jj, uorder=None):
            """One 512-row supertile (jj<4) or the 16-row halo (jj==4)."""
            halo = jj == 4
            nwin = HALO if halo else 512
            n0 = jj * 512
            g = 0 if jj < 2 else 1
            xt = xh if halo else xkj[jj]
            for u in (uorder or range(16)):
                is_l = u < 8
                if is_l and halo:
                    continue                  # halo rows: R only
                ps = ps512.tile([128, 512], F32, tag="ps512", name="ps")
                for ks in range(8):
                    nc.tensor.matmul(ps[:, :nwin], wr_sb[:, u, 128 * ks:128 * ks + 128],
                                     xt[:, ks, :nwin],
                                     start=(ks == 0), stop=(ks == 7))
                for dsub in range(2):
                    src = ps[64 * dsub:64 * dsub + 64, :nwin]
                    bias = br_sb[64 * dsub:64 * dsub + 64, u:u + 1]
                    if is_l:
                        a = 2 * u + dsub
                        dst = et_all[64 * g:64 * g + 64, a,
                                     n0 - 1024 * g:n0 - 1024 * g + nwin]
                        nc.scalar.activation(dst, src, AF.Exp, bias=bias)
                    else:
                        c = 2 * (u - 8) + dsub
                        # g0 chain: blocks 0..8+ ; g1 chain: blocks 8..16+
                        if jj < 2:
                            nc.scalar.activation(rt_all[0:64, c, n0:n0 + 512], src,
                                                 AF.Identity, bias=bias)
                        elif jj == 2:   # blocks 8-11: both chains
                            nc.scalar.activation(rt_all[0:64, c, 1024:RTC],
                                                 ps[64 * dsub:64 * dsub + 64, 0:RTC - 1024],
                                                 AF.Identity, bias=bias)
                            nc.scalar.activation(rt_all[64:128, c, 0:512], src,
                                                 AF.Identity, bias=bias)
                        elif jj == 3:
                            nc.scalar.activation(rt_all[64:128, c, 512:1024], src,
                                                 AF.Identity, bias=bias)
                        else:           # halo block 16
                            nc.scalar.activation(rt_all[64:128, c, 1024:1024 + HALO],
                                                 src, AF.Identity, bias=bias)

        def rt1_copy(lo, hi, eng=None):
            # rt1[:, c, j] = rt[:, c, j+1], via SBUF->SBUF DMA (no engine time;
            # split over c to bound descriptor count)
            nc.sync.dma_start(rt1_all[:, 0:8, lo:hi], rt_all[:, 0:8, lo + 1:hi + 1])
            nc.sync.dma_start(rt1_all[:, 8:15, lo:hi], rt_all[:, 8:15, lo + 1:hi + 1])

        def norm128(w):
            """softmax denom + normalize et over cols [128w, 128w+128)."""
            lo = 128 * w
            e = et_all[:, :, lo:lo + 128]
            nc.vector.tensor_tensor(zts[:], e[:, 0:4, :], e[:, 4:8, :],
                                    op=mybir.AluOpType.add)
            nc.vector.tensor_tensor(zts[:], zts[:], e[:, 8:12, :],
                                    op=mybir.AluOpType.add)
            nc.vector.tensor_tensor(zts[:], zts[:], e[:, 12:16, :],
                                    op=mybir.AluOpType.add)
            nc.vector.tensor_tensor(zts[:, 0:2, :], zts[:, 0:2, :], zts[:, 2:4, :],
                                    op=mybir.AluOpType.add)
            nc.vector.tensor_tensor(zt[:], zts[:, 0, :], zts[:, 1, :],
                                    op=mybir.AluOpType.add)
            nc.vector.reciprocal(zt[:], zt[:])
            nc.vector.tensor_copy(rzb[:], zt[:])
            rz3 = rzb[:].rearrange("p (o n) -> p o n", o=1) \
                .to_broadcast((128, 15, 128))
            ew = et_all[:, 0:15, lo:lo + 128]
            nc.vector.tensor_mul(ew, ew, rz3)

        qts = {}

        def band(cp):
            """Banded MAC for one 128-row chunklet pair -> qt tiles."""
            mp = macp.tile([128, 16, 128], F32, tag="macp", name="mp")
            nc.vector.memset(mp[:, 0, :], 0.0)
            n0 = 128 * cp
            for a in range(15):
                cnt = 15 - a
                p = prodp.tile([128, 15, 128], BF16, tag="prodp", name="p")
                eb = et_all[:, a:a + 1, n0:n0 + 128].to_broadcast((128, cnt, 128))
                eng = nc.gpsimd if a in GP_A else nc.vector
                if a % 2 == 0:
                    # even shift a+1 is 2B-misaligned on rt; use the
                    # pre-shifted rt1 so DVE runs in 2x mode
                    eng.tensor_mul(p[:, 0:cnt, :], eb,
                                   rt1_all[:, 0:cnt, n0 + a:n0 + a + 128])
                else:
                    eng.tensor_mul(p[:, 0:cnt, :], eb,
                                   rt_all[:, 0:cnt, n0 + a + 1:n0 + a + 129])
                for b in range(4):
                    d_lo = max(a + 1, 4 * b)
                    d_hi = 4 * b + 4
                    if d_lo >= d_hi:
                        continue
                    last_a = min(14, 4 * b + 2)
                    nc.tensor.matmul(mp[:, d_lo:d_hi, :], ident[:],
                                     p[:, d_lo - a - 1:d_hi - a - 1, :],
                                     start=(a == 0), stop=(a == last_a))

            for g in range(2):
                qt = qtp.tile([128, 8, 128], BF16, tag="qtp", name="qt")
                for dsub in range(2):
                    csrc = mp[64 * g:64 * g + 64, dsub::2, :]
                    cdst = qt[64 * dsub:64 * dsub + 64, :, :]
                    nc.scalar.copy(cdst, csrc)
                if 8 * g + cp == 15:
                    nc.vector.tensor_mul(qt[:], qt[:], qmask[:])
                qts[(cp, g)] = qt

        def writer(cp):
            """Writer matmuls for chunklet cp (runs one cp behind band)."""
            for g in range(2):
                cb = 8 * g + cp
                qt = qts.pop((cp, g))
                for h in range(2):
                    wp = wps.tile([128, 512], F32, tag="wps", name="wp")
                    for k in range(8):
                        nc.tensor.matmul(wp[:], qt[:, k, :],
                                         ww_sb[:, k, h * 512:(h + 1) * 512],
                                         start=(k == 0), stop=(k == 7))
                    osb = osbp.tile([128, 512], F32, tag="osbp", name="osb")
                    nc.scalar.copy(osb[:], wp[:])
                    nc.sync.dma_start(out_d[cb][:, h * 512:(h + 1) * 512], osb[:])

        # ---------------- interleaved schedule
        reader(0)
        reader(2)
        for w in range(4):
            norm128(w)
        rt1_copy(0, 511, eng=nc.vector)
        reader(1)
        band(0)
        band(1)
        writer(0)
        band(2)
        writer(1)
        # last supertile R-slabs first so rt (and rt1) unblock band(3) early
        reader(3, uorder=list(range(8, 16)))
        rt1_copy(511, 655, eng=nc.vector)
        rt1_copy(655, 1023, eng=nc.vector)
        reader(3, uorder=list(range(0, 8)))
        reader(4)
        band(3)
        writer(2)
        norm128(4)
        band(4)
        writer(3)
        norm128(5)
        band(5)
        writer(4)
        norm128(6)
        band(6)
        writer(5)
        norm128(7)
        rt1_copy(1023, RTC - 1, eng=nc.vector)
        band(7)
        writer(6)
        writer(7)

    nc.compile()
    return nc


def _host_prep(x, W_r, b_r, W_w, b_w):
    """Build the 8 per-core input maps."""
    xf = np.asarray(x, np.float32).reshape(B * N, PD)
    wr = np.asarray(W_r, np.float32).astype(bfnp)
    ww = np.asarray(W_w, np.float32).astype(bfnp)
    br = np.ascontiguousarray(
        np.asarray(b_r, np.float32).reshape(16, 128).T)
    # wr_d[u, p, ks*128+c] = W_r[128*ks + p, 128*u + c]
    wr_t = np.ascontiguousarray(
        wr.reshape(8, 128, 16, 128).transpose(2, 1, 0, 3).reshape(16, 128, 1024))
    ww_t = np.ascontiguousarray(ww.reshape(8, 128, 1024))
    ident = np.eye(128, dtype=np.float32).astype(bfnp)

    in_maps = []
    for c in range(8):
        lo = c * ROWS_PER_CORE
        chunk = np.zeros((NROWS, PD), np.float32)
        chunk[:ROWS_PER_CORE] = xf[lo:lo + ROWS_PER_CORE]
        if c % 2 == 0:
            chunk[ROWS_PER_CORE:] = xf[lo + ROWS_PER_CORE: lo + NROWS]
        # xT[ks, k, n] = chunk[n, 128*ks + k]
        xt = np.ascontiguousarray(
            chunk.astype(bfnp).reshape(NROWS, 8, 128).transpose(1, 2, 0))
        qmask = np.ones((128, 8, 128), np.float32)
        if c % 2 == 1:
            dsub = (np.arange(128)[:, None, None] // 64)
            k = np.arange(8)[None, :, None]
            n = np.arange(128)[None, None, :]
            bad = (n >= 112) & ((n - 112 + 2 * k + dsub) >= 16)
            qmask[np.broadcast_to(bad, (128, 8, 128))] = 0.0
        in_maps.append({
            "xT": xt,
            "wr": wr_t, "ww": ww_t, "br": br,
            "ident": ident, "qmask": qmask.astype(bfnp),
        })
    return in_maps


def kernel(x, W_r, b_r, W_w, b_w):
    if "nc" not in _cache:
        _cache["nc"] = _build()
    nc = _cache["nc"]
    in_maps = _host_prep(x, W_r, b_r, W_w, b_w)
    res = bass_utils.run_bass_kernel_spmd(nc, in_maps, core_ids=list(range(8)))
    out = np.concatenate([r["out"].reshape(ROWS_PER_CORE, ALL)
                          for r in res.results], axis=0)
    out = out.reshape(B, N, ALL).astype(np.float32)
    out += np.asarray(b_w, np.float32).reshape(1, 1, ALL)
    return np.ascontiguousarray(out)
